# revision 58
# baseline (speedup 1.0000x reference)
"""Trainium2 Bass kernel for nn_AdjSMP (2-layer GCN propagation).

out = concat(L2norm(A^2 (x@W_sgc+b)), A^2 s) @ W_last + b_last
with A = D^-1/2 (Adj + I) D^-1/2 on a random graph, N=100k, E=1.6M.

Distribution: nodes row-sharded over 8 cores. The 192-wide propagated
features (h||noise) are built per shard (GEMM on TensorE), all-gathered
in 4 quarter-collectives per layer (so gathers of quarter q overlap the
collective of quarter q+1 and table indices fit int16), and SpMM runs as
dma_gather (512B bf16 rows) + one-hot scatter matmuls into PSUM.

Perf structure (~1.61-1.73 ms measured, vs 4.81 ms for the
single-queue weighted version):
- dma_gather desc-gen is SWDGE ucode pinned to one Q7 cpu pair per
  queue; round-robining windows over all 4 SWDGE queues overlaps
  desc-gen 4x (8.4 -> ~2.5 ns/row), leaving GPSIMD the pacing engine
  at ~1.2 ms with DVE just below it.
- edge weights are separable (w = dinv[row]*dinv[col]), so dinv is
  folded into the node vectors (host prescale of x/s/bias; per-tile
  ACT scales between layers; the final D^-1/2 on h is absorbed by the
  L2 normalization). One-hot matrices are pure 0/1 and built 8 at a
  time with a single DVE is_equal over a broadcast AP.
- y_acc is a bf16 SBUF accumulator; bucket partials flush as an
  identity matmul into PSUM (PE) + copy-out (ACT) instead of a DVE
  tensor add (adds ~1.4e-3 rel err; total 5.5e-3 << 2e-2 gate).
- layer-1 tiles write their y shard and fire quarter AllGathers as
  they finish (inside bucket 3), hiding the inter-layer collective;
  the epilogue is fused per tile into layer-2 completion.
- phase B batches x loads 4 tiles/DMA, copies noise per quarter, and
  spreads dispatch over Sync+ACT sequencers so the first z collective
  fires ~80 us in.
"""
import contextlib
import ctypes
import math
import sys
import types

sys.path.insert(0, "/opt/trn_rl_repo")

import numpy as np
import ml_dtypes

BF16 = ml_dtypes.bfloat16

# ----------------------------------------------------------------------------
# axon NTFF profiling shim (the container's antenv stub lacks axon_hooks)
# ----------------------------------------------------------------------------
_SO_PATH = "/opt/axon/libaxon_pjrt.so"


def _ntff_profile_via_ctypes(so_path):
    lib = ctypes.CDLL(so_path)
    if not hasattr(lib, "axon_start_nrt_profile"):
        return None
    lib.axon_start_nrt_profile.argtypes = [
        ctypes.POINTER(ctypes.c_int64),
        ctypes.c_size_t,
    ]
    lib.axon_start_nrt_profile.restype = ctypes.c_int64
    lib.axon_stop_nrt_profile.argtypes = [ctypes.c_char_p]
    lib.axon_stop_nrt_profile.restype = ctypes.c_int64

    @contextlib.contextmanager
    def _hook(output_dir, device_ids):
        import jax

        jax.devices()
        if device_ids:
            ids = (ctypes.c_int64 * len(device_ids))(*device_ids)
            rc = lib.axon_start_nrt_profile(ids, len(device_ids))
        else:
            rc = lib.axon_start_nrt_profile(None, 0)
        if rc != 0:
            raise RuntimeError(f"axon_start_nrt_profile rc={rc}")
        try:
            yield
        finally:
            n = lib.axon_stop_nrt_profile(str(output_dir).encode())
            if n <= 0:
                print(f"profile: rc={n} for {output_dir}", file=sys.stderr)

    return _hook


def _install_shims():
    if "antenv.axon_hooks" not in sys.modules:
        mod = types.ModuleType("antenv.axon_hooks")
        hook = _ntff_profile_via_ctypes(_SO_PATH)
        mod.get_axon_ntff_profile_hook = lambda: hook
        mod.set_axon_ntff_profile_hook = lambda h: None
        sys.modules["antenv.axon_hooks"] = mod
    from concourse import bass_utils

    bass_utils.upload_artifacts = lambda tmpdir: f"local:{tmpdir}"


_install_shims()

import concourse.bacc as bacc
import concourse.mybir as mybir
import concourse.tile as tile
from concourse.bass_utils import run_bass_kernel_spmd
from concourse.library_config import mlp as mlp_lib

# ----------------------------------------------------------------------------
# configuration
# ----------------------------------------------------------------------------
NCORES = 8
WBATCH = 8       # batches (of 128 edges) per dma_gather window; 1024 idx max
SG = 8           # (batch, tile) one-hot builds per DVE is_equal op


class Cfg:
    def __init__(self, n, d_in, d_h, d_s, d_out):
        assert n % NCORES == 0
        self.N = n
        self.D_IN = d_in      # 256
        self.D_H = d_h        # 128
        self.D_S = d_s        # 64
        self.D_Z = d_h + d_s  # 192 propagated width
        self.D_PAD = 256      # padded bf16 row (512B, multiple of 256B)
        self.D_OUT = d_out    # 40
        self.RPC = n // NCORES
        self.T = math.ceil(self.RPC / 128)   # dest tiles per core
        # small first quarter so the first z/y AllGather completes early
        # and bucket-0 gathers start sooner; rest near-equal
        r128 = lambda v: max(128, (int(v) // 128) * 128)
        q0 = r128(self.RPC * 0.15)
        q1 = r128((self.RPC - q0) / 3)
        self.QS = [q0, q1, q1, self.RPC - q0 - 2 * q1]
        assert self.QS[3] > 0
        self.QOFF = [0, q0, q0 + q1, q0 + 2 * q1]
        self.TQ = [NCORES * s for s in self.QS]   # gather-table rows
        assert max(self.TQ) <= 32767              # int16 gather indices
        self.RPC_PAD = self.T * 128


FULL = Cfg(100000, 256, 128, 64, 40)

LAST_EXEC_NS = None
LAST_RESULTS = None


# ----------------------------------------------------------------------------
# host preprocessing
# ----------------------------------------------------------------------------
def _preprocess(cfg, edge_index):
    """Sort/partition non-self edges; build slot arrays + uniform schedule.

    Weights are separable (w_e = dinv[row]*dinv[col]), so the on-device
    SpMM uses pure 0/1 one-hot matrices: dinv[col] is pre-folded into the
    node vectors (host prescale of x/s, per-tile scales between layers)
    and dinv[row] folded into the layer-output scales. Self loops become
    per-row scales too.
    """
    N = cfg.N
    row0 = np.asarray(edge_index[0])
    col0 = np.asarray(edge_index[1])
    row = np.concatenate([row0, np.arange(N, dtype=row0.dtype)])
    deg = np.bincount(row, minlength=N).astype(np.float32)
    dinv = deg ** np.float32(-0.5)

    selfmask = row0 == col0
    nself = (np.bincount(row0[selfmask], minlength=N) + 1).astype(np.float32)

    row = row0[~selfmask]
    col = col0[~selfmask]

    core = (row // cfg.RPC).astype(np.int32)
    lr = (row % cfg.RPC).astype(np.int32)
    t = lr // 128
    qc = (col % cfg.RPC).astype(np.int32)
    q = np.searchsorted(np.asarray(cfg.QOFF)[1:], qc, side="right")
    q = q.astype(np.int32)
    qs = np.asarray(cfg.QS)[q]
    qoff = np.asarray(cfg.QOFF)[q]
    tab_idx = ((col // cfg.RPC) * qs + (qc - qoff)).astype(np.int16)

    T = cfg.T
    order = np.lexsort((col, t, q, core))
    core_s, q_s, t_s = core[order], q[order], t[order]
    lr_s, tabi_s = lr[order], tab_idx[order]

    # per (core, bucket) counts -> uniform padded bucket sizes
    keycb = core_s.astype(np.int64) * 4 + q_s
    cnt_cb = np.bincount(keycb, minlength=NCORES * 4).reshape(NCORES, 4)
    bslots = (np.ceil(cnt_cb.max(axis=0) / 128).astype(np.int64)) * 128  # [4]
    boff = np.concatenate([[0], np.cumsum(bslots)])
    TOT = int(boff[-1])
    TOTB = TOT // 128

    # slot index per edge: bucket offset + rank within (core, bucket)
    grp_start = np.concatenate([[0], np.cumsum(cnt_cb.reshape(-1))])[:-1]
    rank = np.arange(len(order)) - grp_start[keycb]
    slot = boff[q_s] + rank

    gidx = np.zeros((NCORES, TOT), np.int16)
    idsv = np.zeros((NCORES, TOT), np.int32)
    gidx[core_s, slot] = tabi_s
    idsv[core_s, slot] = lr_s % 128

    # The SPMD schedule must be identical across cores, but per-core tile
    # boundaries within a bucket differ slightly. Per batch we take the
    # UNION of tiles any core touches; per (batch, tile) entry the host
    # emits a masked id column: edges of other tiles get id 255, which
    # never matches iota 0..127, so their S entries are 0.
    sched_batches = []  # per bucket: list of batches: list of (tile, start, stop)
    tid_all = np.full((NCORES, TOT), -1, np.int32)
    tid_all[core_s, slot] = t_s
    ids_bt_cols = []  # list of [NCORES, 128] masked-id columns (bf16)
    for b in range(4):
        lo, hi = int(boff[b]), int(boff[b + 1])
        nb = (hi - lo) // 128
        tb = tid_all[:, lo:hi].reshape(NCORES, nb, 128)
        ib = idsv[:, lo:hi].reshape(NCORES, nb, 128)
        first = {}
        last = {}
        batches = []
        for j in range(nb):
            tl = tb[:, j, :]
            tiles = np.unique(tl[tl >= 0])
            for tt in tiles:
                tt = int(tt)
                if tt not in first:
                    first[tt] = j
                last[tt] = j
            batches.append(sorted(int(x) for x in tiles))
        ann = []
        for j in range(nb):
            ent = []
            for tt in batches[j]:
                m = len(ids_bt_cols)
                col = np.where(tb[:, j, :] == tt, ib[:, j, :], 255)
                ids_bt_cols.append(col.astype(BF16))
                ent.append((tt, first[tt] == j, last[tt] == j, m))
            ann.append(ent)
        sched_batches.append(ann)
    NBT = len(ids_bt_cols)
    ids_bt = np.stack(ids_bt_cols, axis=1)  # [NCORES, NBT, 128]

    def wrap16(a):
        return np.tile(np.ascontiguousarray(a.reshape(-1, 16).T), (8, 1))

    def wrapsw(a):  # [RPC] -> [128, T] f32
        p = np.zeros(cfg.RPC_PAD, np.float32)
        p[:cfg.RPC] = a
        return np.ascontiguousarray(p.reshape(cfg.T, 128).T)

    dinv2 = dinv * dinv
    staged = []
    for c in range(NCORES):
        sl = slice(c * cfg.RPC, (c + 1) * cfg.RPC)
        scales = np.stack([
            wrapsw(nself[sl]),            # layer-1 self-init
            wrapsw(dinv2[sl]),            # phase-D inter-layer scale
            wrapsw(nself[sl] * dinv2[sl]),  # layer-2 self-init
            wrapsw(dinv[sl]),             # epilogue noise scale
        ], axis=1)                        # [128, 4, T]
        staged.append({
            "gidx": wrap16(gidx[c]),
            "ids": np.ascontiguousarray(ids_bt[c].T),   # [128, NBT]
            "scales": np.ascontiguousarray(scales),
        })

    sched = {"batches": sched_batches, "TOT": TOT, "TOTB": TOTB,
             "boff": boff, "NBT": NBT}
    return staged, sched, dinv


# ----------------------------------------------------------------------------
# kernel builder
# ----------------------------------------------------------------------------
def _build(cfg, sched, stop_after="F"):
    TOT, TOTB, NBT = sched["TOT"], sched["TOTB"], sched["NBT"]
    boff = sched["boff"]
    T, RPC = cfg.T, cfg.RPC
    DP, DZ, DH, DS, DO = cfg.D_PAD, cfg.D_Z, cfg.D_H, cfg.D_S, cfg.D_OUT
    bf = mybir.dt.bfloat16
    f32 = mybir.dt.float32

    nc = bacc.Bacc("TRN2", target_bir_lowering=False, debug=False,
                   num_devices=NCORES, num_swdge_queues=4)

    # ---- I/O ----
    xT = nc.dram_tensor("xT", [cfg.D_IN, cfg.RPC_PAD], bf, kind="ExternalInput")
    spk = nc.dram_tensor("spk", [RPC, DP], bf, kind="ExternalInput")
    Wg = nc.dram_tensor("Wg", [cfg.D_IN, DH], bf, kind="ExternalInput")
    bg = nc.dram_tensor("bg", [1, DH], bf, kind="ExternalInput")
    W2a = nc.dram_tensor("W2a", [DH, DO], bf, kind="ExternalInput")
    W2b = nc.dram_tensor("W2b", [DS, DO], bf, kind="ExternalInput")
    b2 = nc.dram_tensor("b2", [1, DO], bf, kind="ExternalInput")
    gidx_d = nc.dram_tensor("gidx", [128, TOT // 16], mybir.dt.int16,
                            kind="ExternalInput")
    ids_d = nc.dram_tensor("ids", [128, NBT], bf, kind="ExternalInput")
    scales_d = nc.dram_tensor("scales", [128, 4, T], f32, kind="ExternalInput")
    iota_d = nc.dram_tensor("iota_c", [128, SG * 128], bf,
                            kind="ExternalInput")
    ident_d = nc.dram_tensor("ident_c", [128, 128], bf, kind="ExternalInput")
    ones_d = nc.dram_tensor("ones_c", [1, 128], bf, kind="ExternalInput")
    dinvr_d = nc.dram_tensor("dinvr", [1, cfg.RPC_PAD], bf,
                             kind="ExternalInput")
    outT = nc.dram_tensor("outT", [DO, RPC], f32, kind="ExternalOutput")

    # ---- internal DRAM ----
    z_shard = [nc.dram_tensor(f"z_shard{q}", [cfg.QS[q], DP], bf)
               for q in range(4)]
    y_shard = [nc.dram_tensor(f"y_shard{q}", [cfg.QS[q], DP], bf)
               for q in range(4)]
    z_tab = [nc.dram_tensor(f"z_tab{q}", [cfg.TQ[q], DP], bf,
                            addr_space="Shared") for q in range(4)]
    y_tab = [nc.dram_tensor(f"y_tab{q}", [cfg.TQ[q], DP], bf,
                            addr_space="Shared") for q in range(4)]

    # ---- resident SBUF ----
    # bf16 accumulator: bucket partials are exact in PSUM f32; the 5
    # bf16 roundings of the running sum add ~0.25% rms, well within the
    # error budget, and let the bucket flush run as an identity-matmul
    # on PE + copy on ACT instead of a tensor add on the saturated DVE.
    y_acc = nc.alloc_sbuf_tensor("y_acc", [128, T * DZ], bf).ap()
    gidx_sb = nc.alloc_sbuf_tensor("gidx_sb", [128, TOT // 16],
                                   mybir.dt.int16).ap()
    ids_sb = nc.alloc_sbuf_tensor("ids_sb", [128, NBT], bf).ap()
    scales_sb = nc.alloc_sbuf_tensor("scales_sb", [128, 4, T], f32).ap()
    iota_sb = nc.alloc_sbuf_tensor("iota_sb", [128, SG, 128], bf).ap()
    ident = nc.alloc_sbuf_tensor("ident", [128, 128], bf).ap()
    ones_k1 = nc.alloc_sbuf_tensor("ones_k1", [1, 128], bf).ap()
    dinvr_sb = nc.alloc_sbuf_tensor("dinvr_sb", [1, cfg.RPC_PAD], bf).ap()
    wg_sb = nc.alloc_sbuf_tensor("wg_sb", [128, 2, DH], bf).ap()
    bg_sb = nc.alloc_sbuf_tensor("bg_sb", [1, DH], bf).ap()
    w2a_sb = nc.alloc_sbuf_tensor("w2a_sb", [DH, DO], bf).ap()
    w2b_sb = nc.alloc_sbuf_tensor("w2b_sb", [DS, DO], bf).ap()
    b2_sb = nc.alloc_sbuf_tensor("b2_sb", [1, DO], bf).ap()

    eq = mybir.AluOpType.is_equal
    mult = mybir.AluOpType.mult
    AF = mybir.ActivationFunctionType

    qof_t = [cfg.QOFF[q] // 128 for q in range(4)] + [T]  # tile idx bounds

    with tile.TileContext(nc) as tc:
        nc.gpsimd.load_library(mlp_lib)

        nc.sync.dma_start(gidx_sb, gidx_d[:])
        nc.sync.dma_start(ids_sb, ids_d[:])
        nc.sync.dma_start(scales_sb, scales_d[:])
        nc.sync.dma_start(dinvr_sb, dinvr_d[:])
        nc.sync.dma_start(wg_sb[:, 0, :], Wg[0:128, :])
        nc.sync.dma_start(wg_sb[:, 1, :], Wg[128:256, :])
        nc.sync.dma_start(bg_sb, bg[:])
        nc.sync.dma_start(w2a_sb, W2a[:])
        nc.sync.dma_start(w2b_sb, W2b[:])
        nc.sync.dma_start(b2_sb, b2[:])
        nc.sync.dma_start(iota_sb, iota_d[:])
        nc.sync.dma_start(ident, ident_d[:])
        nc.sync.dma_start(ones_k1, ones_d[:])

        estack = contextlib.ExitStack()
        with (
            tc.tile_pool(name="g1sb", bufs=12) as g1sb,
            tc.tile_pool(name="gath", bufs=12) as gath,
            tc.tile_pool(name="sp", bufs=6) as sp,
            tc.tile_pool(name="mmps", bufs=4, space="PSUM") as mmps,
            tc.tile_pool(name="io", bufs=3) as iop,
            tc.tile_pool(name="ep", bufs=3) as ep,
        ):
            # PSUM banks: mmps(4) persistent; g1ps(4) lives only during
            # phase B, its banks are reused by the epilogue pools (opened
            # after phase B emission, first used much later)
            g1ps = estack.enter_context(
                tc.tile_pool(name="g1ps", bufs=4, space="PSUM"))
            # ---------------- Phase B: z = [x@Wg + bg ; s], per quarter ----
            for q in range(4 if stop_after >= "B" else 0):
                t0q, t1q = qof_t[q], qof_t[q + 1]
                # noise columns: one DRAM->DRAM copy per quarter
                nc.sync.dma_start(
                    z_shard[q][0:cfg.QS[q], DH:DP],
                    spk[cfg.QOFF[q]:cfg.QOFF[q] + cfg.QS[q], DH:DP])
                for tg in range(t0q, t1q, 4):
                    ng = min(4, t1q - tg)
                    r0g = tg * 128
                    xa = g1sb.tile([128, 4, 128], bf, tag="xa")
                    xb = g1sb.tile([128, 4, 128], bf, tag="xb")
                    nc.sync.dma_start(xa[:, 0:ng, :],
                                      xT[0:128, r0g:r0g + ng * 128])
                    nc.sync.dma_start(xb[:, 0:ng, :],
                                      xT[128:256, r0g:r0g + ng * 128])
                    for ti in range(ng):
                        t = tg + ti
                        r0 = t * 128
                        rows = min(128, RPC - r0)
                        lr0 = r0 - cfg.QOFF[q]
                        ps = g1ps.tile([128, DH], f32, tag="g1")
                        nc.tensor.matmul(ps[:], xa[:, ti, :], wg_sb[:, 0, :],
                                         start=True, stop=False)
                        nc.tensor.matmul(ps[:], xb[:, ti, :], wg_sb[:, 1, :],
                                         start=False, stop=False)
                        nc.tensor.matmul(ps[:], dinvr_sb[:, r0:r0 + 128],
                                         bg_sb, start=False, stop=True)
                        zt = g1sb.tile([128, DH], bf, tag="zt")
                        nc.scalar.copy(zt[:], ps[:])
                        nc.scalar.dma_start(
                            z_shard[q][lr0:lr0 + rows, 0:DH], zt[:rows, :])
                nc.gpsimd.collective_compute(
                    "AllGather", mybir.AluOpType.bypass,
                    replica_groups=[list(range(NCORES))],
                    ins=[z_shard[q][:].opt()],
                    outs=[z_tab[q][:].opt()],
                )

            estack.close()  # free g1ps banks for the epilogue pools
            eps1 = estack.enter_context(
                tc.tile_pool(name="eps1", bufs=1, space="PSUM"))
            eps2 = estack.enter_context(
                tc.tile_pool(name="eps2", bufs=1, space="PSUM"))

            # ---------------- self-loop init: y_acc = nself * z ------------
            def self_init_z():
                for q in range(4):
                    for t in range(qof_t[q], qof_t[q + 1]):
                        r0 = t * 128
                        rows = min(128, RPC - r0)
                        lr0 = r0 - cfg.QOFF[q]
                        zt = iop.tile([128, DZ], bf, tag="zs")
                        if rows < 128:
                            nc.vector.memset(zt[:], 0.0)
                        nc.sync.dma_start(
                            zt[:rows, :], z_shard[q][lr0:lr0 + rows, 0:DZ])
                        nc.scalar.activation(
                            y_acc[:, t * DZ:(t + 1) * DZ], zt[:], AF.Copy,
                            scale=scales_sb[:, 0, t:t + 1])

            # ---------------- epilogue: norm + GEMM2 + out ------------------
            def epilogue(t):
                r0 = t * 128
                rows = min(128, RPC - r0)
                yah = y_acc[:, t * DZ:t * DZ + DH]
                sq = ep.tile([128, DH], bf, tag="sq", name="sq")
                ssq = ep.tile([128, 1], f32, tag="ssq", name="ssq")
                nc.scalar.activation(sq[:], yah, AF.Square, accum_out=ssq[:])
                nc.vector.tensor_scalar_max(ssq[:], ssq[:], 1e-24)
                nrm = ep.tile([128, 1], f32, tag="nrm", name="nrm")
                nc.scalar.activation(nrm[:], ssq[:], AF.Sqrt)
                rinv = ep.tile([128, 1], f32, tag="rinv", name="rinv")
                nc.vector.reciprocal(rinv[:], nrm[:])
                ys = ep.tile([128, DZ], bf, tag="ys", name="ys")
                nc.vector.tensor_scalar_mul(ys[:, 0:DH], yah, rinv[:])
                nc.vector.tensor_scalar_mul(
                    ys[:, DH:DZ], y_acc[:, t * DZ + DH:(t + 1) * DZ],
                    scales_sb[:, 3, t:t + 1])
                pta = eps1.tile([128, 128], bf, tag="pta", name="pta")
                ptb = eps1.tile([64, 128], bf, tag="ptb", name="ptb")
                nc.tensor.transpose(pta[:], ys[:, 0:DH], ident)
                nc.tensor.transpose(ptb[:], ys[:, DH:DZ], ident)
                yTa = ep.tile([128, 128], bf, tag="yTa", name="yTa")
                yTb = ep.tile([64, 128], bf, tag="yTb", name="yTb")
                nc.vector.tensor_copy(yTa[:], pta[:])
                nc.vector.tensor_copy(yTb[:], ptb[:])
                pso = eps2.tile([DO, 128], f32, tag="pso", name="pso")
                nc.tensor.matmul(pso[:], w2a_sb, yTa[:],
                                 start=True, stop=False)
                nc.tensor.matmul(pso[:], w2b_sb, yTb[:],
                                 start=False, stop=False)
                nc.tensor.matmul(pso[:], b2_sb, ones_k1,
                                 start=False, stop=True)
                ot = ep.tile([DO, 128], f32, tag="ot", name="ot")
                nc.scalar.copy(ot[:], pso[:])
                nc.sync.dma_start(outT[:, r0:r0 + rows], ot[:, 0:rows])

            # last bucket (per tile) that touches it, for epilogue timing
            lastb = {}
            for b in range(4):
                for ent in sched["batches"][b]:
                    for (tt, _f, _l, _m) in ent:
                        lastb[tt] = b

            # ---------------- SpMM sweep (both layers) ---------------------
            qrr = [0]

            def spmm(tabs, on_tile_done=None):
                for b in range(4):
                    ann = sched["batches"][b]
                    nb = len(ann)
                    jj0 = int(boff[b]) // 128
                    wi = 0
                    while wi < nb:
                        nbw = min(WBATCH, nb - wi)
                        gw = gath.tile([128, WBATCH, DP], bf, tag="gw")
                        j16 = (jj0 + wi) * 8
                        nc.gpsimd.dma_gather(
                            gw[:, 0:nbw, :], tabs[b][:],
                            gidx_sb[:, j16:j16 + nbw * 8],
                            nbw * 128, nbw * 128, DP,
                            queue_num=qrr[0] % 4)
                        qrr[0] += 1
                        # window's (batch, tile) entries: m's are consecutive
                        ents = [(k, tt, first, last, m)
                                for k in range(nbw)
                                for (tt, first, last, m) in ann[wi + k]]
                        # build 0/1 one-hot stationaries SG at a time
                        sgrp = {}
                        for g0 in range(0, len(ents), SG):
                            grp = ents[g0:g0 + SG]
                            gn = len(grp)
                            m0 = grp[0][4]
                            Sg = sp.tile([128, SG, 128], bf, tag="S")
                            nc.vector.tensor_tensor(
                                Sg[:, 0:gn, :],
                                ids_sb[:, m0:m0 + gn].broadcast_to(
                                    [128, gn, 128]),
                                iota_sb[:, 0:gn, :], eq)
                            for i, e in enumerate(grp):
                                sgrp[e[4]] = (Sg, i)
                        for (k, tt, first, last, m) in ents:
                            if first:
                                cur_ps[tt] = mmps.tile(
                                    [128, DZ], f32, tag="ps", name="ps")
                            ps = cur_ps[tt]
                            Sg, i = sgrp[m]
                            nc.tensor.matmul(
                                ps[:], Sg[:, i, :], gw[:, k, 0:DZ],
                                start=first, stop=False)
                            if last:
                                ya = y_acc[:, tt * DZ:(tt + 1) * DZ]
                                nc.tensor.matmul(ps[:], ident, ya,
                                                 start=False, stop=True)
                                nc.scalar.copy(ya, ps[:])
                                del cur_ps[tt]
                                if (on_tile_done is not None
                                        and lastb[tt] == b):
                                    on_tile_done(tt)
                        wi += nbw

            # ---------------- Phase D (eager, per tile as L1 finishes) -----
            # yb = dinv^2 * y_acc (inter-layer D^-1); then in-place
            # y_acc *= nself*dinv^2 (layer-2 self-loop term); quarter's
            # AllGather fires as soon as its last tile is written.
            tile_q = {}
            for q in range(4):
                for t in range(qof_t[q], qof_t[q + 1]):
                    tile_q[t] = q
            done_cnt = [0, 0, 0, 0]

            def phase_d_tile(t):
                q = tile_q[t]
                r0 = t * 128
                rows = min(128, RPC - r0)
                lr0 = r0 - cfg.QOFF[q]
                ya = y_acc[:, t * DZ:(t + 1) * DZ]
                yb = iop.tile([128, DP], bf, tag="yb")
                nc.scalar.activation(yb[:, 0:DZ], ya, AF.Copy,
                                     scale=scales_sb[:, 1, t:t + 1])
                nc.vector.memset(yb[:, DZ:DP], 0.0)
                nc.sync.dma_start(
                    y_shard[q][lr0:lr0 + rows, :], yb[:rows, :])
                nc.scalar.activation(ya, ya, AF.Copy,
                                     scale=scales_sb[:, 2, t:t + 1])
                done_cnt[q] += 1
                if done_cnt[q] == qof_t[q + 1] - qof_t[q]:
                    nc.gpsimd.collective_compute(
                        "AllGather", mybir.AluOpType.bypass,
                        replica_groups=[list(range(NCORES))],
                        ins=[y_shard[q][:].opt()],
                        outs=[y_tab[q][:].opt()],
                    )

            cur_ps = {}
            if stop_after >= "C":
                self_init_z()
                spmm(z_tab, on_tile_done=phase_d_tile)

            # ---------------- SpMM layer 2 (+ fused epilogue) ---------------
            if stop_after >= "E":
                spmm(y_tab,
                     on_tile_done=epilogue if stop_after >= "F" else None)
            estack.close()

    nc.compile()
    return nc


# ----------------------------------------------------------------------------
# host staging + run
# ----------------------------------------------------------------------------
def _stage_inputs(cfg, x, s, W_sgc, b_sgc, W_last, b_last, staged, dinv):
    in_maps = []
    for c in range(NCORES):
        r0 = c * cfg.RPC
        dv = dinv[r0:r0 + cfg.RPC].astype(np.float32)
        xs = (np.asarray(x[r0:r0 + cfg.RPC], np.float32)
              * dv[:, None]).astype(BF16)
        xTp = np.zeros((cfg.D_IN, cfg.RPC_PAD), BF16)
        xTp[:, :cfg.RPC] = xs.T
        spk = np.zeros((cfg.RPC, cfg.D_PAD), BF16)
        spk[:, cfg.D_H:cfg.D_H + cfg.D_S] = \
            (np.asarray(s[r0:r0 + cfg.RPC], np.float32)
             * dv[:, None]).astype(BF16)
        dvr = np.zeros((1, cfg.RPC_PAD), BF16)
        dvr[0, :cfg.RPC] = dv.astype(BF16)
        m = {
            "xT": np.ascontiguousarray(xTp),
            "spk": spk,
            "Wg": np.asarray(W_sgc).astype(BF16),
            "bg": np.asarray(b_sgc).reshape(1, -1).astype(BF16),
            "W2a": np.asarray(W_last)[:cfg.D_H].astype(BF16),
            "W2b": np.asarray(W_last)[cfg.D_H:].astype(BF16),
            "b2": np.asarray(b_last).reshape(1, -1).astype(BF16),
            "iota_c": np.tile(np.arange(128, dtype=np.float32).astype(BF16),
                              (128, SG)),
            "ident_c": np.eye(128, dtype=np.float32).astype(BF16),
            "ones_c": np.ones((1, 128), BF16),
            "dinvr": dvr,
        }
        m.update(staged[c])
        in_maps.append(m)
    return in_maps


def _run(cfg, inputs, trace=False, stop_after="F"):
    global LAST_EXEC_NS, LAST_RESULTS
    staged, sched, dinv = _preprocess(cfg, inputs["edge_index"])
    nc = _build(cfg, sched, stop_after=stop_after)
    in_maps = _stage_inputs(
        cfg, inputs["x"], inputs["stochastic_feature"], inputs["W_sgc"],
        inputs["b_sgc"], inputs["W_last"], inputs["b_last"], staged, dinv)
    res = run_bass_kernel_spmd(nc, in_maps, list(range(NCORES)), trace=trace)
    LAST_EXEC_NS = res.exec_time_ns
    LAST_RESULTS = res
    out = np.concatenate(
        [res.results[c]["outT"] for c in range(NCORES)], axis=1).T
    return np.ascontiguousarray(out.astype(np.float32))


def kernel(**inputs):
    return _run(FULL, inputs, trace=False)



# revision 59
# speedup vs baseline: 1.1810x; 1.1810x over previous
"""Trainium2 Bass kernel for nn_AdjSMP (2-layer GCN propagation).

out = concat(L2norm(A^2 (x@W_sgc+b)), A^2 s) @ W_last + b_last
with A = D^-1/2 (Adj + I) D^-1/2 on a random graph, N=100k, E=1.6M.

Distribution: nodes row-sharded over 8 cores. The 192-wide propagated
features (h||noise) are built per shard (GEMM on TensorE), all-gathered
in 4 quarter-collectives per layer (so gathers of quarter q overlap the
collective of quarter q+1 and table indices fit int16), and SpMM runs as
dma_gather (512B bf16 rows) + one-hot scatter matmuls into PSUM.

Perf structure (~1.61-1.73 ms measured, vs 4.81 ms for the
single-queue weighted version):
- dma_gather desc-gen is SWDGE ucode pinned to one Q7 cpu pair per
  queue; round-robining windows over all 4 SWDGE queues overlaps
  desc-gen 4x (8.4 -> ~2.5 ns/row), leaving GPSIMD the pacing engine
  at ~1.2 ms with DVE just below it.
- edge weights are separable (w = dinv[row]*dinv[col]), so dinv is
  folded into the node vectors (host prescale of x/s/bias; per-tile
  ACT scales between layers; the final D^-1/2 on h is absorbed by the
  L2 normalization). One-hot matrices are pure 0/1 and built 8 at a
  time with a single DVE is_equal over a broadcast AP.
- y_acc is a bf16 SBUF accumulator; bucket partials flush as an
  identity matmul into PSUM (PE) + copy-out (ACT) instead of a DVE
  tensor add (adds ~1.4e-3 rel err; total 5.5e-3 << 2e-2 gate).
- layer-1 tiles write their y shard and fire quarter AllGathers as
  they finish (inside bucket 3), hiding the inter-layer collective;
  the epilogue is fused per tile into layer-2 completion.
- phase B batches x loads 4 tiles/DMA, copies noise per quarter, and
  spreads dispatch over Sync+ACT sequencers so the first z collective
  fires ~80 us in.
"""
import contextlib
import ctypes
import math
import sys
import types

sys.path.insert(0, "/opt/trn_rl_repo")

import numpy as np
import ml_dtypes

BF16 = ml_dtypes.bfloat16

# ----------------------------------------------------------------------------
# axon NTFF profiling shim (the container's antenv stub lacks axon_hooks)
# ----------------------------------------------------------------------------
_SO_PATH = "/opt/axon/libaxon_pjrt.so"


def _ntff_profile_via_ctypes(so_path):
    lib = ctypes.CDLL(so_path)
    if not hasattr(lib, "axon_start_nrt_profile"):
        return None
    lib.axon_start_nrt_profile.argtypes = [
        ctypes.POINTER(ctypes.c_int64),
        ctypes.c_size_t,
    ]
    lib.axon_start_nrt_profile.restype = ctypes.c_int64
    lib.axon_stop_nrt_profile.argtypes = [ctypes.c_char_p]
    lib.axon_stop_nrt_profile.restype = ctypes.c_int64

    @contextlib.contextmanager
    def _hook(output_dir, device_ids):
        import jax

        jax.devices()
        if device_ids:
            ids = (ctypes.c_int64 * len(device_ids))(*device_ids)
            rc = lib.axon_start_nrt_profile(ids, len(device_ids))
        else:
            rc = lib.axon_start_nrt_profile(None, 0)
        if rc != 0:
            raise RuntimeError(f"axon_start_nrt_profile rc={rc}")
        try:
            yield
        finally:
            n = lib.axon_stop_nrt_profile(str(output_dir).encode())
            if n <= 0:
                print(f"profile: rc={n} for {output_dir}", file=sys.stderr)

    return _hook


def _install_shims():
    if "antenv.axon_hooks" not in sys.modules:
        mod = types.ModuleType("antenv.axon_hooks")
        hook = _ntff_profile_via_ctypes(_SO_PATH)
        mod.get_axon_ntff_profile_hook = lambda: hook
        mod.set_axon_ntff_profile_hook = lambda h: None
        sys.modules["antenv.axon_hooks"] = mod
    from concourse import bass_utils

    bass_utils.upload_artifacts = lambda tmpdir: f"local:{tmpdir}"


_install_shims()

import concourse.bacc as bacc
import concourse.mybir as mybir
import concourse.tile as tile
from concourse.bass_utils import run_bass_kernel_spmd
from concourse.library_config import mlp as mlp_lib

# ----------------------------------------------------------------------------
# configuration
# ----------------------------------------------------------------------------
NCORES = 8
WBATCH = 8       # batches (of 128 edges) per dma_gather window; 1024 idx max
SG = 8           # (batch, tile) one-hot builds per DVE is_equal op


class Cfg:
    def __init__(self, n, d_in, d_h, d_s, d_out):
        assert n % NCORES == 0
        self.N = n
        self.D_IN = d_in      # 256
        self.D_H = d_h        # 128
        self.D_S = d_s        # 64
        self.D_Z = d_h + d_s  # 192 propagated width
        self.D_PAD = 256      # padded bf16 row (512B, multiple of 256B)
        self.D_OUT = d_out    # 40
        self.RPC = n // NCORES
        self.T = math.ceil(self.RPC / 128)   # dest tiles per core
        # small first quarter so the first z/y AllGather completes early
        # and bucket-0 gathers start sooner; rest near-equal
        r128 = lambda v: max(128, (int(v) // 128) * 128)
        q0 = r128(self.RPC * 0.15)
        q1 = r128((self.RPC - q0) / 3)
        self.QS = [q0, q1, q1, self.RPC - q0 - 2 * q1]
        assert self.QS[3] > 0
        self.QOFF = [0, q0, q0 + q1, q0 + 2 * q1]
        self.TQ = [NCORES * s for s in self.QS]   # gather-table rows
        assert max(self.TQ) <= 32767              # int16 gather indices
        self.RPC_PAD = self.T * 128


FULL = Cfg(100000, 256, 128, 64, 40)

LAST_EXEC_NS = None
LAST_RESULTS = None


# ----------------------------------------------------------------------------
# host preprocessing
# ----------------------------------------------------------------------------
def _preprocess(cfg, edge_index):
    """Sort/partition non-self edges; build slot arrays + uniform schedule.

    Weights are separable (w_e = dinv[row]*dinv[col]), so the on-device
    SpMM uses pure 0/1 one-hot matrices: dinv[col] is pre-folded into the
    node vectors (host prescale of x/s, per-tile scales between layers)
    and dinv[row] folded into the layer-output scales. Self loops become
    per-row scales too.
    """
    N = cfg.N
    row0 = np.asarray(edge_index[0])
    col0 = np.asarray(edge_index[1])
    row = np.concatenate([row0, np.arange(N, dtype=row0.dtype)])
    deg = np.bincount(row, minlength=N).astype(np.float32)
    dinv = deg ** np.float32(-0.5)

    selfmask = row0 == col0
    nself = (np.bincount(row0[selfmask], minlength=N) + 1).astype(np.float32)

    row = row0[~selfmask]
    col = col0[~selfmask]

    core = (row // cfg.RPC).astype(np.int32)
    lr = (row % cfg.RPC).astype(np.int32)
    t = lr // 128
    qc = (col % cfg.RPC).astype(np.int32)
    q = np.searchsorted(np.asarray(cfg.QOFF)[1:], qc, side="right")
    q = q.astype(np.int32)
    qs = np.asarray(cfg.QS)[q]
    qoff = np.asarray(cfg.QOFF)[q]
    tab_idx = ((col // cfg.RPC) * qs + (qc - qoff)).astype(np.int16)

    T = cfg.T
    order = np.lexsort((col, t, q, core))
    core_s, q_s, t_s = core[order], q[order], t[order]
    lr_s, tabi_s = lr[order], tab_idx[order]

    # per (core, bucket) counts -> uniform padded bucket sizes
    keycb = core_s.astype(np.int64) * 4 + q_s
    cnt_cb = np.bincount(keycb, minlength=NCORES * 4).reshape(NCORES, 4)
    bslots = (np.ceil(cnt_cb.max(axis=0) / 128).astype(np.int64)) * 128  # [4]
    boff = np.concatenate([[0], np.cumsum(bslots)])
    TOT = int(boff[-1])
    TOTB = TOT // 128

    # slot index per edge: bucket offset + rank within (core, bucket)
    grp_start = np.concatenate([[0], np.cumsum(cnt_cb.reshape(-1))])[:-1]
    rank = np.arange(len(order)) - grp_start[keycb]
    slot = boff[q_s] + rank

    gidx = np.zeros((NCORES, TOT), np.int16)
    idsv = np.zeros((NCORES, TOT), np.int32)
    gidx[core_s, slot] = tabi_s
    idsv[core_s, slot] = lr_s % 128

    # The SPMD schedule must be identical across cores, but per-core tile
    # boundaries within a bucket differ slightly. Per batch we take the
    # UNION of tiles any core touches; per (batch, tile) entry the host
    # emits a masked id column: edges of other tiles get id 255, which
    # never matches iota 0..127, so their S entries are 0.
    sched_batches = []  # per bucket: list of batches: list of (tile, start, stop)
    tid_all = np.full((NCORES, TOT), -1, np.int32)
    tid_all[core_s, slot] = t_s
    ids_bt_cols = []  # list of [NCORES, 128] masked-id columns (bf16)
    for b in range(4):
        lo, hi = int(boff[b]), int(boff[b + 1])
        nb = (hi - lo) // 128
        tb = tid_all[:, lo:hi].reshape(NCORES, nb, 128)
        ib = idsv[:, lo:hi].reshape(NCORES, nb, 128)
        first = {}
        last = {}
        batches = []
        for j in range(nb):
            tl = tb[:, j, :]
            tiles = np.unique(tl[tl >= 0])
            for tt in tiles:
                tt = int(tt)
                if tt not in first:
                    first[tt] = j
                last[tt] = j
            batches.append(sorted(int(x) for x in tiles))
        ann = []
        for j in range(nb):
            ent = []
            for tt in batches[j]:
                m = len(ids_bt_cols)
                col = np.where(tb[:, j, :] == tt, ib[:, j, :], 255)
                ids_bt_cols.append(col.astype(BF16))
                ent.append((tt, first[tt] == j, last[tt] == j, m))
            ann.append(ent)
        sched_batches.append(ann)
    NBT = len(ids_bt_cols)
    ids_bt = np.stack(ids_bt_cols, axis=1)  # [NCORES, NBT, 128]

    def wrap16(a):
        return np.tile(np.ascontiguousarray(a.reshape(-1, 16).T), (8, 1))

    def wrapsw(a):  # [RPC] -> [128, T] f32
        p = np.zeros(cfg.RPC_PAD, np.float32)
        p[:cfg.RPC] = a
        return np.ascontiguousarray(p.reshape(cfg.T, 128).T)

    dinv2 = dinv * dinv
    staged = []
    for c in range(NCORES):
        sl = slice(c * cfg.RPC, (c + 1) * cfg.RPC)
        scales = np.stack([
            wrapsw(nself[sl]),            # layer-1 self-init
            wrapsw(dinv2[sl]),            # phase-D inter-layer scale
            wrapsw(nself[sl] * dinv2[sl]),  # layer-2 self-init
            wrapsw(dinv[sl]),             # epilogue noise scale
        ], axis=1)                        # [128, 4, T]
        staged.append({
            "gidx": wrap16(gidx[c]),
            "ids": np.ascontiguousarray(ids_bt[c].T),   # [128, NBT]
            "scales": np.ascontiguousarray(scales),
        })

    sched = {"batches": sched_batches, "TOT": TOT, "TOTB": TOTB,
             "boff": boff, "NBT": NBT}
    return staged, sched, dinv


# ----------------------------------------------------------------------------
# kernel builder
# ----------------------------------------------------------------------------
def _build(cfg, sched, stop_after="F"):
    TOT, TOTB, NBT = sched["TOT"], sched["TOTB"], sched["NBT"]
    boff = sched["boff"]
    T, RPC = cfg.T, cfg.RPC
    DP, DZ, DH, DS, DO = cfg.D_PAD, cfg.D_Z, cfg.D_H, cfg.D_S, cfg.D_OUT
    bf = mybir.dt.bfloat16
    f32 = mybir.dt.float32

    nc = bacc.Bacc("TRN2", target_bir_lowering=False, debug=False,
                   num_devices=NCORES, num_swdge_queues=4)

    # ---- I/O ----
    xT = nc.dram_tensor("xT", [cfg.D_IN, cfg.RPC_PAD], bf, kind="ExternalInput")
    spk = nc.dram_tensor("spk", [RPC, DP], bf, kind="ExternalInput")
    Wg = nc.dram_tensor("Wg", [cfg.D_IN, DH], bf, kind="ExternalInput")
    bg = nc.dram_tensor("bg", [1, DH], bf, kind="ExternalInput")
    W2a = nc.dram_tensor("W2a", [DH, DO], bf, kind="ExternalInput")
    W2b = nc.dram_tensor("W2b", [DS, DO], bf, kind="ExternalInput")
    b2 = nc.dram_tensor("b2", [1, DO], bf, kind="ExternalInput")
    gidx_d = nc.dram_tensor("gidx", [128, TOT // 16], mybir.dt.int16,
                            kind="ExternalInput")
    ids_d = nc.dram_tensor("ids", [128, NBT], bf, kind="ExternalInput")
    scales_d = nc.dram_tensor("scales", [128, 4, T], f32, kind="ExternalInput")
    iota_d = nc.dram_tensor("iota_c", [128, SG * 128], bf,
                            kind="ExternalInput")
    ident_d = nc.dram_tensor("ident_c", [128, 128], bf, kind="ExternalInput")
    ones_d = nc.dram_tensor("ones_c", [1, 128], bf, kind="ExternalInput")
    dinvr_d = nc.dram_tensor("dinvr", [1, cfg.RPC_PAD], bf,
                             kind="ExternalInput")
    outT = nc.dram_tensor("outT", [DO, RPC], f32, kind="ExternalOutput")

    # ---- internal DRAM ----
    z_shard = [nc.dram_tensor(f"z_shard{q}", [cfg.QS[q], DP], bf)
               for q in range(4)]
    y_shard = [nc.dram_tensor(f"y_shard{q}", [cfg.QS[q], DP], bf)
               for q in range(4)]
    z_tab = [nc.dram_tensor(f"z_tab{q}", [cfg.TQ[q], DP], bf,
                            addr_space="Shared") for q in range(4)]
    y_tab = [nc.dram_tensor(f"y_tab{q}", [cfg.TQ[q], DP], bf,
                            addr_space="Shared") for q in range(4)]

    # ---- resident SBUF ----
    # bf16 accumulator: bucket partials are exact in PSUM f32; the 5
    # bf16 roundings of the running sum add ~0.25% rms, well within the
    # error budget, and let the bucket flush run as an identity-matmul
    # on PE + copy on ACT instead of a tensor add on the saturated DVE.
    y_acc = nc.alloc_sbuf_tensor("y_acc", [128, T * DZ], bf).ap()
    gidx_sb = nc.alloc_sbuf_tensor("gidx_sb", [128, TOT // 16],
                                   mybir.dt.int16).ap()
    ids_sb = nc.alloc_sbuf_tensor("ids_sb", [128, NBT], bf).ap()
    scales_sb = nc.alloc_sbuf_tensor("scales_sb", [128, 4, T], f32).ap()
    iota_sb = nc.alloc_sbuf_tensor("iota_sb", [128, SG, 128], bf).ap()
    ident = nc.alloc_sbuf_tensor("ident", [128, 128], bf).ap()
    ones_k1 = nc.alloc_sbuf_tensor("ones_k1", [1, 128], bf).ap()
    dinvr_sb = nc.alloc_sbuf_tensor("dinvr_sb", [1, cfg.RPC_PAD], bf).ap()
    wg_sb = nc.alloc_sbuf_tensor("wg_sb", [128, 2, DH], bf).ap()
    bg_sb = nc.alloc_sbuf_tensor("bg_sb", [1, DH], bf).ap()
    w2a_sb = nc.alloc_sbuf_tensor("w2a_sb", [DH, DO], bf).ap()
    w2b_sb = nc.alloc_sbuf_tensor("w2b_sb", [DS, DO], bf).ap()
    b2_sb = nc.alloc_sbuf_tensor("b2_sb", [1, DO], bf).ap()

    eq = mybir.AluOpType.is_equal
    mult = mybir.AluOpType.mult
    AF = mybir.ActivationFunctionType

    qof_t = [cfg.QOFF[q] // 128 for q in range(4)] + [T]  # tile idx bounds

    with tile.TileContext(nc) as tc:
        nc.gpsimd.load_library(mlp_lib)

        nc.sync.dma_start(gidx_sb, gidx_d[:])
        nc.sync.dma_start(ids_sb, ids_d[:])
        nc.sync.dma_start(scales_sb, scales_d[:])
        nc.sync.dma_start(dinvr_sb, dinvr_d[:])
        nc.sync.dma_start(wg_sb[:, 0, :], Wg[0:128, :])
        nc.sync.dma_start(wg_sb[:, 1, :], Wg[128:256, :])
        nc.sync.dma_start(bg_sb, bg[:])
        nc.sync.dma_start(w2a_sb, W2a[:])
        nc.sync.dma_start(w2b_sb, W2b[:])
        nc.sync.dma_start(b2_sb, b2[:])
        nc.sync.dma_start(iota_sb, iota_d[:])
        nc.sync.dma_start(ident, ident_d[:])
        nc.sync.dma_start(ones_k1, ones_d[:])

        estack = contextlib.ExitStack()
        with (
            tc.tile_pool(name="g1sb", bufs=12) as g1sb,
            tc.tile_pool(name="gath", bufs=12) as gath,
            tc.tile_pool(name="sp", bufs=6) as sp,
            tc.tile_pool(name="mmps", bufs=4, space="PSUM") as mmps,
            tc.tile_pool(name="io", bufs=3) as iop,
            tc.tile_pool(name="ep", bufs=3) as ep,
        ):
            # PSUM banks: mmps(4) persistent; g1ps(4) lives only during
            # phase B, its banks are reused by the epilogue pools (opened
            # after phase B emission, first used much later)
            g1ps = estack.enter_context(
                tc.tile_pool(name="g1ps", bufs=4, space="PSUM"))
            # ---------------- Phase B: z = [x@Wg + bg ; s], per quarter ----
            for q in range(4 if stop_after >= "B" else 0):
                t0q, t1q = qof_t[q], qof_t[q + 1]
                # noise columns: one DRAM->DRAM copy per quarter
                nc.sync.dma_start(
                    z_shard[q][0:cfg.QS[q], DH:DP],
                    spk[cfg.QOFF[q]:cfg.QOFF[q] + cfg.QS[q], DH:DP])
                for tg in range(t0q, t1q, 4):
                    ng = min(4, t1q - tg)
                    r0g = tg * 128
                    xa = g1sb.tile([128, 4, 128], bf, tag="xa")
                    xb = g1sb.tile([128, 4, 128], bf, tag="xb")
                    nc.sync.dma_start(xa[:, 0:ng, :],
                                      xT[0:128, r0g:r0g + ng * 128])
                    nc.sync.dma_start(xb[:, 0:ng, :],
                                      xT[128:256, r0g:r0g + ng * 128])
                    for ti in range(ng):
                        t = tg + ti
                        r0 = t * 128
                        rows = min(128, RPC - r0)
                        lr0 = r0 - cfg.QOFF[q]
                        ps = g1ps.tile([128, DH], f32, tag="g1")
                        nc.tensor.matmul(ps[:], xa[:, ti, :], wg_sb[:, 0, :],
                                         start=True, stop=False)
                        nc.tensor.matmul(ps[:], xb[:, ti, :], wg_sb[:, 1, :],
                                         start=False, stop=False)
                        nc.tensor.matmul(ps[:], dinvr_sb[:, r0:r0 + 128],
                                         bg_sb, start=False, stop=True)
                        zt = g1sb.tile([128, DH], bf, tag="zt")
                        nc.scalar.copy(zt[:], ps[:])
                        nc.scalar.dma_start(
                            z_shard[q][lr0:lr0 + rows, 0:DH], zt[:rows, :])
                nc.gpsimd.collective_compute(
                    "AllGather", mybir.AluOpType.bypass,
                    replica_groups=[list(range(NCORES))],
                    ins=[z_shard[q][:].opt()],
                    outs=[z_tab[q][:].opt()],
                )

            estack.close()  # free g1ps banks for the epilogue pools
            eps1 = estack.enter_context(
                tc.tile_pool(name="eps1", bufs=1, space="PSUM"))
            eps2 = estack.enter_context(
                tc.tile_pool(name="eps2", bufs=1, space="PSUM"))

            # ---------------- self-loop init: y_acc = nself * z ------------
            def self_init_z():
                for q in range(4):
                    for t in range(qof_t[q], qof_t[q + 1]):
                        r0 = t * 128
                        rows = min(128, RPC - r0)
                        lr0 = r0 - cfg.QOFF[q]
                        zt = iop.tile([128, DZ], bf, tag="zs")
                        if rows < 128:
                            nc.vector.memset(zt[:], 0.0)
                        nc.sync.dma_start(
                            zt[:rows, :], z_shard[q][lr0:lr0 + rows, 0:DZ])
                        nc.scalar.activation(
                            y_acc[:, t * DZ:(t + 1) * DZ], zt[:], AF.Copy,
                            scale=scales_sb[:, 0, t:t + 1])

            # ---------------- epilogue: norm + GEMM2 + out ------------------
            def epilogue(t):
                r0 = t * 128
                rows = min(128, RPC - r0)
                yah = y_acc[:, t * DZ:t * DZ + DH]
                sq = ep.tile([128, DH], bf, tag="sq", name="sq")
                ssq = ep.tile([128, 1], f32, tag="ssq", name="ssq")
                nc.scalar.activation(sq[:], yah, AF.Square, accum_out=ssq[:])
                nc.vector.tensor_scalar_max(ssq[:], ssq[:], 1e-24)
                nrm = ep.tile([128, 1], f32, tag="nrm", name="nrm")
                nc.scalar.activation(nrm[:], ssq[:], AF.Sqrt)
                rinv = ep.tile([128, 1], f32, tag="rinv", name="rinv")
                nc.vector.reciprocal(rinv[:], nrm[:])
                ys = ep.tile([128, DZ], bf, tag="ys", name="ys")
                nc.scalar.activation(ys[:, 0:DH], yah, AF.Copy,
                                     scale=rinv[:])
                nc.scalar.activation(ys[:, DH:DZ],
                                     y_acc[:, t * DZ + DH:(t + 1) * DZ],
                                     AF.Copy, scale=scales_sb[:, 3, t:t + 1])
                pta = eps1.tile([128, 128], bf, tag="pta", name="pta")
                ptb = eps1.tile([64, 128], bf, tag="ptb", name="ptb")
                nc.tensor.transpose(pta[:], ys[:, 0:DH], ident)
                nc.tensor.transpose(ptb[:], ys[:, DH:DZ], ident)
                yTa = ep.tile([128, 128], bf, tag="yTa", name="yTa")
                yTb = ep.tile([64, 128], bf, tag="yTb", name="yTb")
                nc.vector.tensor_copy(yTa[:], pta[:])
                nc.vector.tensor_copy(yTb[:], ptb[:])
                pso = eps2.tile([DO, 128], f32, tag="pso", name="pso")
                nc.tensor.matmul(pso[:], w2a_sb, yTa[:],
                                 start=True, stop=False)
                nc.tensor.matmul(pso[:], w2b_sb, yTb[:],
                                 start=False, stop=False)
                nc.tensor.matmul(pso[:], b2_sb, ones_k1,
                                 start=False, stop=True)
                ot = ep.tile([DO, 128], f32, tag="ot", name="ot")
                nc.scalar.copy(ot[:], pso[:])
                nc.sync.dma_start(outT[:, r0:r0 + rows], ot[:, 0:rows])

            # last bucket (per tile) that touches it, for epilogue timing
            lastb = {}
            for b in range(4):
                for ent in sched["batches"][b]:
                    for (tt, _f, _l, _m) in ent:
                        lastb[tt] = b

            # ---------------- SpMM sweep (both layers) ---------------------
            qrr = [0]

            def spmm(tabs, on_tile_done=None):
                for b in range(4):
                    ann = sched["batches"][b]
                    nb = len(ann)
                    jj0 = int(boff[b]) // 128
                    wi = 0
                    while wi < nb:
                        nbw = min(WBATCH, nb - wi)
                        gw = gath.tile([128, WBATCH, DP], bf, tag="gw")
                        j16 = (jj0 + wi) * 8
                        nc.gpsimd.dma_gather(
                            gw[:, 0:nbw, :], tabs[b][:],
                            gidx_sb[:, j16:j16 + nbw * 8],
                            nbw * 128, nbw * 128, DP,
                            queue_num=qrr[0] % 4)
                        qrr[0] += 1
                        # window's (batch, tile) entries: m's are consecutive
                        ents = [(k, tt, first, last, m)
                                for k in range(nbw)
                                for (tt, first, last, m) in ann[wi + k]]
                        # build 0/1 one-hot stationaries SG at a time
                        sgrp = {}
                        for g0 in range(0, len(ents), SG):
                            grp = ents[g0:g0 + SG]
                            gn = len(grp)
                            m0 = grp[0][4]
                            Sg = sp.tile([128, SG, 128], bf, tag="S")
                            nc.vector.tensor_tensor(
                                Sg[:, 0:gn, :],
                                ids_sb[:, m0:m0 + gn].broadcast_to(
                                    [128, gn, 128]),
                                iota_sb[:, 0:gn, :], eq)
                            for i, e in enumerate(grp):
                                sgrp[e[4]] = (Sg, i)
                        for (k, tt, first, last, m) in ents:
                            if first:
                                cur_ps[tt] = mmps.tile(
                                    [128, DZ], f32, tag="ps", name="ps")
                            ps = cur_ps[tt]
                            Sg, i = sgrp[m]
                            nc.tensor.matmul(
                                ps[:], Sg[:, i, :], gw[:, k, 0:DZ],
                                start=first, stop=False)
                            if last:
                                ya = y_acc[:, tt * DZ:(tt + 1) * DZ]
                                nc.tensor.matmul(ps[:], ident, ya,
                                                 start=False, stop=True)
                                nc.scalar.copy(ya, ps[:])
                                del cur_ps[tt]
                                if (on_tile_done is not None
                                        and lastb[tt] == b):
                                    on_tile_done(tt)
                        wi += nbw

            # ---------------- Phase D (eager, per tile as L1 finishes) -----
            # yb = dinv^2 * y_acc (inter-layer D^-1); then in-place
            # y_acc *= nself*dinv^2 (layer-2 self-loop term); quarter's
            # AllGather fires as soon as its last tile is written.
            tile_q = {}
            for q in range(4):
                for t in range(qof_t[q], qof_t[q + 1]):
                    tile_q[t] = q
            done_cnt = [0, 0, 0, 0]

            def phase_d_tile(t):
                q = tile_q[t]
                r0 = t * 128
                rows = min(128, RPC - r0)
                lr0 = r0 - cfg.QOFF[q]
                ya = y_acc[:, t * DZ:(t + 1) * DZ]
                yb = iop.tile([128, DP], bf, tag="yb")
                nc.scalar.activation(yb[:, 0:DZ], ya, AF.Copy,
                                     scale=scales_sb[:, 1, t:t + 1])
                nc.vector.memset(yb[:, DZ:DP], 0.0)
                nc.sync.dma_start(
                    y_shard[q][lr0:lr0 + rows, :], yb[:rows, :])
                nc.scalar.activation(ya, ya, AF.Copy,
                                     scale=scales_sb[:, 2, t:t + 1])
                done_cnt[q] += 1
                if done_cnt[q] == qof_t[q + 1] - qof_t[q]:
                    nc.gpsimd.collective_compute(
                        "AllGather", mybir.AluOpType.bypass,
                        replica_groups=[list(range(NCORES))],
                        ins=[y_shard[q][:].opt()],
                        outs=[y_tab[q][:].opt()],
                    )

            cur_ps = {}
            if stop_after >= "C":
                self_init_z()
                spmm(z_tab, on_tile_done=phase_d_tile)

            # ---------------- SpMM layer 2 (+ fused epilogue) ---------------
            if stop_after >= "E":
                spmm(y_tab,
                     on_tile_done=epilogue if stop_after >= "F" else None)
            estack.close()

    nc.compile()
    return nc


# ----------------------------------------------------------------------------
# host staging + run
# ----------------------------------------------------------------------------
def _stage_inputs(cfg, x, s, W_sgc, b_sgc, W_last, b_last, staged, dinv):
    in_maps = []
    for c in range(NCORES):
        r0 = c * cfg.RPC
        dv = dinv[r0:r0 + cfg.RPC].astype(np.float32)
        xs = (np.asarray(x[r0:r0 + cfg.RPC], np.float32)
              * dv[:, None]).astype(BF16)
        xTp = np.zeros((cfg.D_IN, cfg.RPC_PAD), BF16)
        xTp[:, :cfg.RPC] = xs.T
        spk = np.zeros((cfg.RPC, cfg.D_PAD), BF16)
        spk[:, cfg.D_H:cfg.D_H + cfg.D_S] = \
            (np.asarray(s[r0:r0 + cfg.RPC], np.float32)
             * dv[:, None]).astype(BF16)
        dvr = np.zeros((1, cfg.RPC_PAD), BF16)
        dvr[0, :cfg.RPC] = dv.astype(BF16)
        m = {
            "xT": np.ascontiguousarray(xTp),
            "spk": spk,
            "Wg": np.asarray(W_sgc).astype(BF16),
            "bg": np.asarray(b_sgc).reshape(1, -1).astype(BF16),
            "W2a": np.asarray(W_last)[:cfg.D_H].astype(BF16),
            "W2b": np.asarray(W_last)[cfg.D_H:].astype(BF16),
            "b2": np.asarray(b_last).reshape(1, -1).astype(BF16),
            "iota_c": np.tile(np.arange(128, dtype=np.float32).astype(BF16),
                              (128, SG)),
            "ident_c": np.eye(128, dtype=np.float32).astype(BF16),
            "ones_c": np.ones((1, 128), BF16),
            "dinvr": dvr,
        }
        m.update(staged[c])
        in_maps.append(m)
    return in_maps


def _run(cfg, inputs, trace=False, stop_after="F"):
    global LAST_EXEC_NS, LAST_RESULTS
    staged, sched, dinv = _preprocess(cfg, inputs["edge_index"])
    nc = _build(cfg, sched, stop_after=stop_after)
    in_maps = _stage_inputs(
        cfg, inputs["x"], inputs["stochastic_feature"], inputs["W_sgc"],
        inputs["b_sgc"], inputs["W_last"], inputs["b_last"], staged, dinv)
    res = run_bass_kernel_spmd(nc, in_maps, list(range(NCORES)), trace=trace)
    LAST_EXEC_NS = res.exec_time_ns
    LAST_RESULTS = res
    out = np.concatenate(
        [res.results[c]["outT"] for c in range(NCORES)], axis=1).T
    return np.ascontiguousarray(out.astype(np.float32))


def kernel(**inputs):
    return _run(FULL, inputs, trace=False)



# revision 61
# speedup vs baseline: 1.1819x; 1.0007x over previous
"""Trainium2 Bass kernel for nn_AdjSMP (2-layer GCN propagation).

out = concat(L2norm(A^2 (x@W_sgc+b)), A^2 s) @ W_last + b_last
with A = D^-1/2 (Adj + I) D^-1/2 on a random graph, N=100k, E=1.6M.

Distribution: nodes row-sharded over 8 cores. The 192-wide propagated
features (h||noise) are built per shard (GEMM on TensorE), all-gathered
in 4 quarter-collectives per layer (so gathers of quarter q overlap the
collective of quarter q+1 and table indices fit int16), and SpMM runs as
dma_gather (512B bf16 rows) + one-hot scatter matmuls into PSUM.

Perf structure (~1.61-1.73 ms measured, vs 4.81 ms for the
single-queue weighted version):
- dma_gather desc-gen is SWDGE ucode pinned to one Q7 cpu pair per
  queue; round-robining windows over all 4 SWDGE queues overlaps
  desc-gen 4x (8.4 -> ~2.5 ns/row), leaving GPSIMD the pacing engine
  at ~1.2 ms with DVE just below it.
- edge weights are separable (w = dinv[row]*dinv[col]), so dinv is
  folded into the node vectors (host prescale of x/s/bias; per-tile
  ACT scales between layers; the final D^-1/2 on h is absorbed by the
  L2 normalization). One-hot matrices are pure 0/1 and built 8 at a
  time with a single DVE is_equal over a broadcast AP.
- y_acc is a bf16 SBUF accumulator; bucket partials flush as an
  identity matmul into PSUM (PE) + copy-out (ACT) instead of a DVE
  tensor add (adds ~1.4e-3 rel err; total 5.5e-3 << 2e-2 gate).
- layer-1 tiles write their y shard and fire quarter AllGathers as
  they finish (inside bucket 3), hiding the inter-layer collective;
  the epilogue is fused per tile into layer-2 completion.
- phase B batches x loads 4 tiles/DMA, copies noise per quarter, and
  spreads dispatch over Sync+ACT sequencers so the first z collective
  fires ~80 us in.
"""
import contextlib
import ctypes
import math
import sys
import types

sys.path.insert(0, "/opt/trn_rl_repo")

import numpy as np
import ml_dtypes

BF16 = ml_dtypes.bfloat16

# ----------------------------------------------------------------------------
# axon NTFF profiling shim (the container's antenv stub lacks axon_hooks)
# ----------------------------------------------------------------------------
_SO_PATH = "/opt/axon/libaxon_pjrt.so"


def _ntff_profile_via_ctypes(so_path):
    lib = ctypes.CDLL(so_path)
    if not hasattr(lib, "axon_start_nrt_profile"):
        return None
    lib.axon_start_nrt_profile.argtypes = [
        ctypes.POINTER(ctypes.c_int64),
        ctypes.c_size_t,
    ]
    lib.axon_start_nrt_profile.restype = ctypes.c_int64
    lib.axon_stop_nrt_profile.argtypes = [ctypes.c_char_p]
    lib.axon_stop_nrt_profile.restype = ctypes.c_int64

    @contextlib.contextmanager
    def _hook(output_dir, device_ids):
        import jax

        jax.devices()
        if device_ids:
            ids = (ctypes.c_int64 * len(device_ids))(*device_ids)
            rc = lib.axon_start_nrt_profile(ids, len(device_ids))
        else:
            rc = lib.axon_start_nrt_profile(None, 0)
        if rc != 0:
            raise RuntimeError(f"axon_start_nrt_profile rc={rc}")
        try:
            yield
        finally:
            n = lib.axon_stop_nrt_profile(str(output_dir).encode())
            if n <= 0:
                print(f"profile: rc={n} for {output_dir}", file=sys.stderr)

    return _hook


def _install_shims():
    if "antenv.axon_hooks" not in sys.modules:
        mod = types.ModuleType("antenv.axon_hooks")
        hook = _ntff_profile_via_ctypes(_SO_PATH)
        mod.get_axon_ntff_profile_hook = lambda: hook
        mod.set_axon_ntff_profile_hook = lambda h: None
        sys.modules["antenv.axon_hooks"] = mod
    from concourse import bass_utils

    bass_utils.upload_artifacts = lambda tmpdir: f"local:{tmpdir}"


_install_shims()

import concourse.bacc as bacc
import concourse.mybir as mybir
import concourse.tile as tile
from concourse.bass_utils import run_bass_kernel_spmd
from concourse.library_config import mlp as mlp_lib

# ----------------------------------------------------------------------------
# configuration
# ----------------------------------------------------------------------------
NCORES = 8
WBATCH = 8       # batches (of 128 edges) per dma_gather window; 1024 idx max
SG = 16          # (batch, tile) one-hot builds per DVE is_equal op


class Cfg:
    def __init__(self, n, d_in, d_h, d_s, d_out):
        assert n % NCORES == 0
        self.N = n
        self.D_IN = d_in      # 256
        self.D_H = d_h        # 128
        self.D_S = d_s        # 64
        self.D_Z = d_h + d_s  # 192 propagated width
        self.D_PAD = 256      # padded bf16 row (512B, multiple of 256B)
        self.D_OUT = d_out    # 40
        self.RPC = n // NCORES
        self.T = math.ceil(self.RPC / 128)   # dest tiles per core
        # small first quarter so the first z/y AllGather completes early
        # and bucket-0 gathers start sooner; rest near-equal
        r128 = lambda v: max(128, (int(v) // 128) * 128)
        q0 = r128(self.RPC * 0.15)
        q1 = r128((self.RPC - q0) / 3)
        self.QS = [q0, q1, q1, self.RPC - q0 - 2 * q1]
        assert self.QS[3] > 0
        self.QOFF = [0, q0, q0 + q1, q0 + 2 * q1]
        self.TQ = [NCORES * s for s in self.QS]   # gather-table rows
        assert max(self.TQ) <= 32767              # int16 gather indices
        self.RPC_PAD = self.T * 128


FULL = Cfg(100000, 256, 128, 64, 40)

LAST_EXEC_NS = None
LAST_RESULTS = None


# ----------------------------------------------------------------------------
# host preprocessing
# ----------------------------------------------------------------------------
def _preprocess(cfg, edge_index):
    """Sort/partition non-self edges; build slot arrays + uniform schedule.

    Weights are separable (w_e = dinv[row]*dinv[col]), so the on-device
    SpMM uses pure 0/1 one-hot matrices: dinv[col] is pre-folded into the
    node vectors (host prescale of x/s, per-tile scales between layers)
    and dinv[row] folded into the layer-output scales. Self loops become
    per-row scales too.
    """
    N = cfg.N
    row0 = np.asarray(edge_index[0])
    col0 = np.asarray(edge_index[1])
    row = np.concatenate([row0, np.arange(N, dtype=row0.dtype)])
    deg = np.bincount(row, minlength=N).astype(np.float32)
    dinv = deg ** np.float32(-0.5)

    selfmask = row0 == col0
    nself = (np.bincount(row0[selfmask], minlength=N) + 1).astype(np.float32)

    row = row0[~selfmask]
    col = col0[~selfmask]

    core = (row // cfg.RPC).astype(np.int32)
    lr = (row % cfg.RPC).astype(np.int32)
    t = lr // 128
    qc = (col % cfg.RPC).astype(np.int32)
    q = np.searchsorted(np.asarray(cfg.QOFF)[1:], qc, side="right")
    q = q.astype(np.int32)
    qs = np.asarray(cfg.QS)[q]
    qoff = np.asarray(cfg.QOFF)[q]
    tab_idx = ((col // cfg.RPC) * qs + (qc - qoff)).astype(np.int16)

    T = cfg.T
    order = np.lexsort((col, t, q, core))
    core_s, q_s, t_s = core[order], q[order], t[order]
    lr_s, tabi_s = lr[order], tab_idx[order]

    # per (core, bucket) counts -> uniform padded bucket sizes
    keycb = core_s.astype(np.int64) * 4 + q_s
    cnt_cb = np.bincount(keycb, minlength=NCORES * 4).reshape(NCORES, 4)
    bslots = (np.ceil(cnt_cb.max(axis=0) / 128).astype(np.int64)) * 128  # [4]
    boff = np.concatenate([[0], np.cumsum(bslots)])
    TOT = int(boff[-1])
    TOTB = TOT // 128

    # slot index per edge: bucket offset + rank within (core, bucket)
    grp_start = np.concatenate([[0], np.cumsum(cnt_cb.reshape(-1))])[:-1]
    rank = np.arange(len(order)) - grp_start[keycb]
    slot = boff[q_s] + rank

    gidx = np.zeros((NCORES, TOT), np.int16)
    idsv = np.zeros((NCORES, TOT), np.int32)
    gidx[core_s, slot] = tabi_s
    idsv[core_s, slot] = lr_s % 128

    # The SPMD schedule must be identical across cores, but per-core tile
    # boundaries within a bucket differ slightly. Per batch we take the
    # UNION of tiles any core touches; per (batch, tile) entry the host
    # emits a masked id column: edges of other tiles get id 255, which
    # never matches iota 0..127, so their S entries are 0.
    sched_batches = []  # per bucket: list of batches: list of (tile, start, stop)
    tid_all = np.full((NCORES, TOT), -1, np.int32)
    tid_all[core_s, slot] = t_s
    ids_bt_cols = []  # list of [NCORES, 128] masked-id columns (bf16)
    for b in range(4):
        lo, hi = int(boff[b]), int(boff[b + 1])
        nb = (hi - lo) // 128
        tb = tid_all[:, lo:hi].reshape(NCORES, nb, 128)
        ib = idsv[:, lo:hi].reshape(NCORES, nb, 128)
        first = {}
        last = {}
        batches = []
        for j in range(nb):
            tl = tb[:, j, :]
            tiles = np.unique(tl[tl >= 0])
            for tt in tiles:
                tt = int(tt)
                if tt not in first:
                    first[tt] = j
                last[tt] = j
            batches.append(sorted(int(x) for x in tiles))
        ann = []
        for j in range(nb):
            ent = []
            for tt in batches[j]:
                m = len(ids_bt_cols)
                col = np.where(tb[:, j, :] == tt, ib[:, j, :], 255)
                ids_bt_cols.append(col.astype(BF16))
                ent.append((tt, first[tt] == j, last[tt] == j, m))
            ann.append(ent)
        sched_batches.append(ann)
    NBT = len(ids_bt_cols)
    ids_bt = np.stack(ids_bt_cols, axis=1)  # [NCORES, NBT, 128]

    def wrap16(a):
        return np.tile(np.ascontiguousarray(a.reshape(-1, 16).T), (8, 1))

    def wrapsw(a):  # [RPC] -> [128, T] f32
        p = np.zeros(cfg.RPC_PAD, np.float32)
        p[:cfg.RPC] = a
        return np.ascontiguousarray(p.reshape(cfg.T, 128).T)

    dinv2 = dinv * dinv
    staged = []
    for c in range(NCORES):
        sl = slice(c * cfg.RPC, (c + 1) * cfg.RPC)
        scales = np.stack([
            wrapsw(nself[sl]),            # layer-1 self-init
            wrapsw(dinv2[sl]),            # phase-D inter-layer scale
            wrapsw(nself[sl] * dinv2[sl]),  # layer-2 self-init
            wrapsw(dinv[sl]),             # epilogue noise scale
        ], axis=1)                        # [128, 4, T]
        staged.append({
            "gidx": wrap16(gidx[c]),
            "ids": np.ascontiguousarray(ids_bt[c].T),   # [128, NBT]
            "scales": np.ascontiguousarray(scales),
        })

    sched = {"batches": sched_batches, "TOT": TOT, "TOTB": TOTB,
             "boff": boff, "NBT": NBT}
    return staged, sched, dinv


# ----------------------------------------------------------------------------
# kernel builder
# ----------------------------------------------------------------------------
def _build(cfg, sched, stop_after="F"):
    TOT, TOTB, NBT = sched["TOT"], sched["TOTB"], sched["NBT"]
    boff = sched["boff"]
    T, RPC = cfg.T, cfg.RPC
    DP, DZ, DH, DS, DO = cfg.D_PAD, cfg.D_Z, cfg.D_H, cfg.D_S, cfg.D_OUT
    bf = mybir.dt.bfloat16
    f32 = mybir.dt.float32

    nc = bacc.Bacc("TRN2", target_bir_lowering=False, debug=False,
                   num_devices=NCORES, num_swdge_queues=4)

    # ---- I/O ----
    xT = nc.dram_tensor("xT", [cfg.D_IN, cfg.RPC_PAD], bf, kind="ExternalInput")
    spk = nc.dram_tensor("spk", [RPC, DP], bf, kind="ExternalInput")
    Wg = nc.dram_tensor("Wg", [cfg.D_IN, DH], bf, kind="ExternalInput")
    bg = nc.dram_tensor("bg", [1, DH], bf, kind="ExternalInput")
    W2a = nc.dram_tensor("W2a", [DH, DO], bf, kind="ExternalInput")
    W2b = nc.dram_tensor("W2b", [DS, DO], bf, kind="ExternalInput")
    b2 = nc.dram_tensor("b2", [1, DO], bf, kind="ExternalInput")
    gidx_d = nc.dram_tensor("gidx", [128, TOT // 16], mybir.dt.int16,
                            kind="ExternalInput")
    ids_d = nc.dram_tensor("ids", [128, NBT], bf, kind="ExternalInput")
    scales_d = nc.dram_tensor("scales", [128, 4, T], f32, kind="ExternalInput")
    iota_d = nc.dram_tensor("iota_c", [128, SG * 128], bf,
                            kind="ExternalInput")
    ident_d = nc.dram_tensor("ident_c", [128, 128], bf, kind="ExternalInput")
    ones_d = nc.dram_tensor("ones_c", [1, 128], bf, kind="ExternalInput")
    dinvr_d = nc.dram_tensor("dinvr", [1, cfg.RPC_PAD], bf,
                             kind="ExternalInput")
    outT = nc.dram_tensor("outT", [DO, RPC], f32, kind="ExternalOutput")

    # ---- internal DRAM ----
    z_shard = [nc.dram_tensor(f"z_shard{q}", [cfg.QS[q], DP], bf)
               for q in range(4)]
    y_shard = [nc.dram_tensor(f"y_shard{q}", [cfg.QS[q], DP], bf)
               for q in range(4)]
    z_tab = [nc.dram_tensor(f"z_tab{q}", [cfg.TQ[q], DP], bf,
                            addr_space="Shared") for q in range(4)]
    y_tab = [nc.dram_tensor(f"y_tab{q}", [cfg.TQ[q], DP], bf,
                            addr_space="Shared") for q in range(4)]

    # ---- resident SBUF ----
    # bf16 accumulator: bucket partials are exact in PSUM f32; the 5
    # bf16 roundings of the running sum add ~0.25% rms, well within the
    # error budget, and let the bucket flush run as an identity-matmul
    # on PE + copy on ACT instead of a tensor add on the saturated DVE.
    y_acc = nc.alloc_sbuf_tensor("y_acc", [128, T * DZ], bf).ap()
    gidx_sb = nc.alloc_sbuf_tensor("gidx_sb", [128, TOT // 16],
                                   mybir.dt.int16).ap()
    ids_sb = nc.alloc_sbuf_tensor("ids_sb", [128, NBT], bf).ap()
    scales_sb = nc.alloc_sbuf_tensor("scales_sb", [128, 4, T], f32).ap()
    iota_sb = nc.alloc_sbuf_tensor("iota_sb", [128, SG, 128], bf).ap()
    ident = nc.alloc_sbuf_tensor("ident", [128, 128], bf).ap()
    ones_k1 = nc.alloc_sbuf_tensor("ones_k1", [1, 128], bf).ap()
    dinvr_sb = nc.alloc_sbuf_tensor("dinvr_sb", [1, cfg.RPC_PAD], bf).ap()
    wg_sb = nc.alloc_sbuf_tensor("wg_sb", [128, 2, DH], bf).ap()
    bg_sb = nc.alloc_sbuf_tensor("bg_sb", [1, DH], bf).ap()
    w2a_sb = nc.alloc_sbuf_tensor("w2a_sb", [DH, DO], bf).ap()
    w2b_sb = nc.alloc_sbuf_tensor("w2b_sb", [DS, DO], bf).ap()
    b2_sb = nc.alloc_sbuf_tensor("b2_sb", [1, DO], bf).ap()

    eq = mybir.AluOpType.is_equal
    mult = mybir.AluOpType.mult
    AF = mybir.ActivationFunctionType

    qof_t = [cfg.QOFF[q] // 128 for q in range(4)] + [T]  # tile idx bounds

    with tile.TileContext(nc) as tc:
        nc.gpsimd.load_library(mlp_lib)

        nc.sync.dma_start(gidx_sb, gidx_d[:])
        nc.sync.dma_start(ids_sb, ids_d[:])
        nc.sync.dma_start(scales_sb, scales_d[:])
        nc.sync.dma_start(dinvr_sb, dinvr_d[:])
        nc.sync.dma_start(wg_sb[:, 0, :], Wg[0:128, :])
        nc.sync.dma_start(wg_sb[:, 1, :], Wg[128:256, :])
        nc.sync.dma_start(bg_sb, bg[:])
        nc.sync.dma_start(w2a_sb, W2a[:])
        nc.sync.dma_start(w2b_sb, W2b[:])
        nc.sync.dma_start(b2_sb, b2[:])
        nc.sync.dma_start(iota_sb, iota_d[:])
        nc.sync.dma_start(ident, ident_d[:])
        nc.sync.dma_start(ones_k1, ones_d[:])

        estack = contextlib.ExitStack()
        with (
            tc.tile_pool(name="g1sb", bufs=12) as g1sb,
            tc.tile_pool(name="gath", bufs=12) as gath,
            tc.tile_pool(name="sp", bufs=4) as sp,
            tc.tile_pool(name="mmps", bufs=4, space="PSUM") as mmps,
            tc.tile_pool(name="io", bufs=3) as iop,
            tc.tile_pool(name="ep", bufs=3) as ep,
        ):
            # PSUM banks: mmps(4) persistent; g1ps(4) lives only during
            # phase B, its banks are reused by the epilogue pools (opened
            # after phase B emission, first used much later)
            g1ps = estack.enter_context(
                tc.tile_pool(name="g1ps", bufs=4, space="PSUM"))
            # ---------------- Phase B: z = [x@Wg + bg ; s], per quarter ----
            for q in range(4 if stop_after >= "B" else 0):
                t0q, t1q = qof_t[q], qof_t[q + 1]
                # noise columns: one DRAM->DRAM copy per quarter
                nc.sync.dma_start(
                    z_shard[q][0:cfg.QS[q], DH:DP],
                    spk[cfg.QOFF[q]:cfg.QOFF[q] + cfg.QS[q], DH:DP])
                for tg in range(t0q, t1q, 4):
                    ng = min(4, t1q - tg)
                    r0g = tg * 128
                    xa = g1sb.tile([128, 4, 128], bf, tag="xa")
                    xb = g1sb.tile([128, 4, 128], bf, tag="xb")
                    nc.sync.dma_start(xa[:, 0:ng, :],
                                      xT[0:128, r0g:r0g + ng * 128])
                    nc.sync.dma_start(xb[:, 0:ng, :],
                                      xT[128:256, r0g:r0g + ng * 128])
                    for ti in range(ng):
                        t = tg + ti
                        r0 = t * 128
                        rows = min(128, RPC - r0)
                        lr0 = r0 - cfg.QOFF[q]
                        ps = g1ps.tile([128, DH], f32, tag="g1")
                        nc.tensor.matmul(ps[:], xa[:, ti, :], wg_sb[:, 0, :],
                                         start=True, stop=False)
                        nc.tensor.matmul(ps[:], xb[:, ti, :], wg_sb[:, 1, :],
                                         start=False, stop=False)
                        nc.tensor.matmul(ps[:], dinvr_sb[:, r0:r0 + 128],
                                         bg_sb, start=False, stop=True)
                        zt = g1sb.tile([128, DH], bf, tag="zt")
                        nc.scalar.copy(zt[:], ps[:])
                        nc.scalar.dma_start(
                            z_shard[q][lr0:lr0 + rows, 0:DH], zt[:rows, :])
                nc.gpsimd.collective_compute(
                    "AllGather", mybir.AluOpType.bypass,
                    replica_groups=[list(range(NCORES))],
                    ins=[z_shard[q][:].opt()],
                    outs=[z_tab[q][:].opt()],
                )

            estack.close()  # free g1ps banks for the epilogue pools
            eps1 = estack.enter_context(
                tc.tile_pool(name="eps1", bufs=1, space="PSUM"))
            eps2 = estack.enter_context(
                tc.tile_pool(name="eps2", bufs=1, space="PSUM"))

            # ---------------- self-loop init: y_acc = nself * z ------------
            def self_init_z():
                for q in range(4):
                    for t in range(qof_t[q], qof_t[q + 1]):
                        r0 = t * 128
                        rows = min(128, RPC - r0)
                        lr0 = r0 - cfg.QOFF[q]
                        zt = iop.tile([128, DZ], bf, tag="zs")
                        if rows < 128:
                            nc.vector.memset(zt[:], 0.0)
                        nc.sync.dma_start(
                            zt[:rows, :], z_shard[q][lr0:lr0 + rows, 0:DZ])
                        nc.scalar.activation(
                            y_acc[:, t * DZ:(t + 1) * DZ], zt[:], AF.Copy,
                            scale=scales_sb[:, 0, t:t + 1])

            # ---------------- epilogue: norm + GEMM2 + out ------------------
            def epilogue(t):
                r0 = t * 128
                rows = min(128, RPC - r0)
                yah = y_acc[:, t * DZ:t * DZ + DH]
                sq = ep.tile([128, DH], bf, tag="sq", name="sq")
                ssq = ep.tile([128, 1], f32, tag="ssq", name="ssq")
                nc.scalar.activation(sq[:], yah, AF.Square, accum_out=ssq[:])
                nc.vector.tensor_scalar_max(ssq[:], ssq[:], 1e-24)
                nrm = ep.tile([128, 1], f32, tag="nrm", name="nrm")
                nc.scalar.activation(nrm[:], ssq[:], AF.Sqrt)
                rinv = ep.tile([128, 1], f32, tag="rinv", name="rinv")
                nc.vector.reciprocal(rinv[:], nrm[:])
                ys = ep.tile([128, DZ], bf, tag="ys", name="ys")
                nc.scalar.activation(ys[:, 0:DH], yah, AF.Copy,
                                     scale=rinv[:])
                nc.scalar.activation(ys[:, DH:DZ],
                                     y_acc[:, t * DZ + DH:(t + 1) * DZ],
                                     AF.Copy, scale=scales_sb[:, 3, t:t + 1])
                pta = eps1.tile([128, 128], bf, tag="pta", name="pta")
                ptb = eps1.tile([64, 128], bf, tag="ptb", name="ptb")
                nc.tensor.transpose(pta[:], ys[:, 0:DH], ident)
                nc.tensor.transpose(ptb[:], ys[:, DH:DZ], ident)
                yTa = ep.tile([128, 128], bf, tag="yTa", name="yTa")
                yTb = ep.tile([64, 128], bf, tag="yTb", name="yTb")
                nc.vector.tensor_copy(yTa[:], pta[:])
                nc.vector.tensor_copy(yTb[:], ptb[:])
                pso = eps2.tile([DO, 128], f32, tag="pso", name="pso")
                nc.tensor.matmul(pso[:], w2a_sb, yTa[:],
                                 start=True, stop=False)
                nc.tensor.matmul(pso[:], w2b_sb, yTb[:],
                                 start=False, stop=False)
                nc.tensor.matmul(pso[:], b2_sb, ones_k1,
                                 start=False, stop=True)
                ot = ep.tile([DO, 128], f32, tag="ot", name="ot")
                nc.scalar.copy(ot[:], pso[:])
                nc.sync.dma_start(outT[:, r0:r0 + rows], ot[:, 0:rows])

            # last bucket (per tile) that touches it, for epilogue timing
            lastb = {}
            for b in range(4):
                for ent in sched["batches"][b]:
                    for (tt, _f, _l, _m) in ent:
                        lastb[tt] = b

            # ---------------- SpMM sweep (both layers) ---------------------
            qrr = [0]

            def spmm(tabs, on_tile_done=None):
                for b in range(4):
                    ann = sched["batches"][b]
                    nb = len(ann)
                    jj0 = int(boff[b]) // 128
                    wi = 0
                    while wi < nb:
                        nbw = min(WBATCH, nb - wi)
                        gw = gath.tile([128, WBATCH, DP], bf, tag="gw")
                        j16 = (jj0 + wi) * 8
                        nc.gpsimd.dma_gather(
                            gw[:, 0:nbw, :], tabs[b][:],
                            gidx_sb[:, j16:j16 + nbw * 8],
                            nbw * 128, nbw * 128, DP,
                            queue_num=qrr[0] % 4)
                        qrr[0] += 1
                        # window's (batch, tile) entries: m's are consecutive
                        ents = [(k, tt, first, last, m)
                                for k in range(nbw)
                                for (tt, first, last, m) in ann[wi + k]]
                        # build 0/1 one-hot stationaries SG at a time
                        sgrp = {}
                        for g0 in range(0, len(ents), SG):
                            grp = ents[g0:g0 + SG]
                            gn = len(grp)
                            m0 = grp[0][4]
                            Sg = sp.tile([128, SG, 128], bf, tag="S")
                            nc.vector.tensor_tensor(
                                Sg[:, 0:gn, :],
                                ids_sb[:, m0:m0 + gn].broadcast_to(
                                    [128, gn, 128]),
                                iota_sb[:, 0:gn, :], eq)
                            for i, e in enumerate(grp):
                                sgrp[e[4]] = (Sg, i)
                        for (k, tt, first, last, m) in ents:
                            if first:
                                cur_ps[tt] = mmps.tile(
                                    [128, DZ], f32, tag="ps", name="ps")
                            ps = cur_ps[tt]
                            Sg, i = sgrp[m]
                            nc.tensor.matmul(
                                ps[:], Sg[:, i, :], gw[:, k, 0:DZ],
                                start=first, stop=False)
                            if last:
                                ya = y_acc[:, tt * DZ:(tt + 1) * DZ]
                                nc.tensor.matmul(ps[:], ident, ya,
                                                 start=False, stop=True)
                                nc.scalar.copy(ya, ps[:])
                                del cur_ps[tt]
                                if (on_tile_done is not None
                                        and lastb[tt] == b):
                                    on_tile_done(tt)
                        wi += nbw

            # ---------------- Phase D (eager, per tile as L1 finishes) -----
            # yb = dinv^2 * y_acc (inter-layer D^-1); then in-place
            # y_acc *= nself*dinv^2 (layer-2 self-loop term); quarter's
            # AllGather fires as soon as its last tile is written.
            tile_q = {}
            for q in range(4):
                for t in range(qof_t[q], qof_t[q + 1]):
                    tile_q[t] = q
            done_cnt = [0, 0, 0, 0]

            def phase_d_tile(t):
                q = tile_q[t]
                r0 = t * 128
                rows = min(128, RPC - r0)
                lr0 = r0 - cfg.QOFF[q]
                ya = y_acc[:, t * DZ:(t + 1) * DZ]
                yb = iop.tile([128, DP], bf, tag="yb")
                nc.scalar.activation(yb[:, 0:DZ], ya, AF.Copy,
                                     scale=scales_sb[:, 1, t:t + 1])
                nc.vector.memset(yb[:, DZ:DP], 0.0)
                nc.sync.dma_start(
                    y_shard[q][lr0:lr0 + rows, :], yb[:rows, :])
                nc.scalar.activation(ya, ya, AF.Copy,
                                     scale=scales_sb[:, 2, t:t + 1])
                done_cnt[q] += 1
                if done_cnt[q] == qof_t[q + 1] - qof_t[q]:
                    nc.gpsimd.collective_compute(
                        "AllGather", mybir.AluOpType.bypass,
                        replica_groups=[list(range(NCORES))],
                        ins=[y_shard[q][:].opt()],
                        outs=[y_tab[q][:].opt()],
                    )

            cur_ps = {}
            if stop_after >= "C":
                self_init_z()
                spmm(z_tab, on_tile_done=phase_d_tile)

            # ---------------- SpMM layer 2 (+ fused epilogue) ---------------
            if stop_after >= "E":
                spmm(y_tab,
                     on_tile_done=epilogue if stop_after >= "F" else None)
            estack.close()

    nc.compile()
    return nc


# ----------------------------------------------------------------------------
# host staging + run
# ----------------------------------------------------------------------------
def _stage_inputs(cfg, x, s, W_sgc, b_sgc, W_last, b_last, staged, dinv):
    in_maps = []
    for c in range(NCORES):
        r0 = c * cfg.RPC
        dv = dinv[r0:r0 + cfg.RPC].astype(np.float32)
        xs = (np.asarray(x[r0:r0 + cfg.RPC], np.float32)
              * dv[:, None]).astype(BF16)
        xTp = np.zeros((cfg.D_IN, cfg.RPC_PAD), BF16)
        xTp[:, :cfg.RPC] = xs.T
        spk = np.zeros((cfg.RPC, cfg.D_PAD), BF16)
        spk[:, cfg.D_H:cfg.D_H + cfg.D_S] = \
            (np.asarray(s[r0:r0 + cfg.RPC], np.float32)
             * dv[:, None]).astype(BF16)
        dvr = np.zeros((1, cfg.RPC_PAD), BF16)
        dvr[0, :cfg.RPC] = dv.astype(BF16)
        m = {
            "xT": np.ascontiguousarray(xTp),
            "spk": spk,
            "Wg": np.asarray(W_sgc).astype(BF16),
            "bg": np.asarray(b_sgc).reshape(1, -1).astype(BF16),
            "W2a": np.asarray(W_last)[:cfg.D_H].astype(BF16),
            "W2b": np.asarray(W_last)[cfg.D_H:].astype(BF16),
            "b2": np.asarray(b_last).reshape(1, -1).astype(BF16),
            "iota_c": np.tile(np.arange(128, dtype=np.float32).astype(BF16),
                              (128, SG)),
            "ident_c": np.eye(128, dtype=np.float32).astype(BF16),
            "ones_c": np.ones((1, 128), BF16),
            "dinvr": dvr,
        }
        m.update(staged[c])
        in_maps.append(m)
    return in_maps


def _run(cfg, inputs, trace=False, stop_after="F"):
    global LAST_EXEC_NS, LAST_RESULTS
    staged, sched, dinv = _preprocess(cfg, inputs["edge_index"])
    nc = _build(cfg, sched, stop_after=stop_after)
    in_maps = _stage_inputs(
        cfg, inputs["x"], inputs["stochastic_feature"], inputs["W_sgc"],
        inputs["b_sgc"], inputs["W_last"], inputs["b_last"], staged, dinv)
    res = run_bass_kernel_spmd(nc, in_maps, list(range(NCORES)), trace=trace)
    LAST_EXEC_NS = res.exec_time_ns
    LAST_RESULTS = res
    out = np.concatenate(
        [res.results[c]["outT"] for c in range(NCORES)], axis=1).T
    return np.ascontiguousarray(out.astype(np.float32))


def kernel(**inputs):
    return _run(FULL, inputs, trace=False)

